# revision 5
# baseline (speedup 1.0000x reference)
"""Distributed multi-head attention kernel for 8 Trainium2 NeuronCores.

Problem: y = softmax((x Wq^T)(x Wk^T)^T / sqrt(D)) (x Wv^T) Wo^T + bo
with B=4, T=2048, C=1280, H=20, D=64, float32 I/O.

Sharding (sequence parallel, rank independent):
  Each core owns a T/8 token slice of all 4 batches (1024 tokens).
  It computes Q/K/V projections for its tokens, AllGathers K^T and V
  (bf16) across the 8 cores, runs full attention for its queries over
  the gathered keys/values of the matching batch, and applies the
  output projection for its tokens. Host reassembles the T axis.

Attention is computed in a transposed "S_T[k, q]" layout so that the
softmax denominator comes out of the same matmul that computes P@V
(a row of ones appended to V gives sum_k P in psum row 64), avoiding
partition-axis reductions. exp() runs on ScalarE from large staged
SBUF tiles to amortize the per-instruction ACT overhead.

Compute dtype is bf16 (fp32 matmul is 4x slower on the PE array);
accumulation is fp32 in PSUM. I/O stays fp32.
"""

import os
import sys
import types

import numpy as np
import ml_dtypes

import concourse.bass as bass
import concourse.mybir as mybir
import concourse.tile as tile
from concourse import bacc
from concourse.bass_utils import run_bass_kernel_spmd

N_CORES = 8
C = 1280
H = 20
D = 64
B = 4
CT = C // 128  # 10 c-tiles
BF = mybir.dt.bfloat16
F32 = mybir.dt.float32
SCALE = 1.0 / (D ** 0.5)

LAST_EXEC_TIME_NS = None
_BUILD_CACHE = {}


def _install_ntff_hook():
    """The trimmed antenv package lacks axon_hooks; register the NTFF
    profile hook by hand so trace=True can time the NEFF on silicon.
    Safe no-op if anything is missing."""
    if "antenv.axon_hooks" in sys.modules:
        return
    try:
        from trn_agent_boot.trn_boot import _ntff_profile_via_ctypes

        hook = _ntff_profile_via_ctypes("/opt/axon/libaxon_pjrt.so")
        mod = types.ModuleType("antenv.axon_hooks")
        mod.get_axon_ntff_profile_hook = lambda: hook
        mod.set_axon_ntff_profile_hook = lambda h: None
        sys.modules["antenv.axon_hooks"] = mod
        import antenv

        antenv.axon_hooks = mod
    except Exception:
        pass


def _chunks(total, step):
    out = []
    o = 0
    while o < total:
        out.append((o, min(step, total - o)))
        o += step
    return out


def build(T):
    """Build the SPMD Bass graph for sequence length T (full, unsharded)."""
    TS = T // N_CORES          # tokens per batch per core
    TOK = B * TS               # tokens per core
    KT = T // 128              # 128-wide key tiles per batch
    assert TS % 128 == 0, "key tiles must not cross rank chunks"
    assert TS <= 512, "q tile must fit one matmul moving operand"

    nc = bacc.Bacc("TRN2", target_bir_lowering=False, debug=False,
                   num_devices=N_CORES)

    xT = nc.dram_tensor("xT", [C, TOK], BF, kind="ExternalInput").ap()
    wqT = nc.dram_tensor("wqT", [C, C], BF, kind="ExternalInput").ap()
    wkT = nc.dram_tensor("wkT", [C, C], BF, kind="ExternalInput").ap()
    wvT = nc.dram_tensor("wvT", [C, C], BF, kind="ExternalInput").ap()
    woT = nc.dram_tensor("woT", [C, C], BF, kind="ExternalInput").ap()
    bo_d = nc.dram_tensor("bo", [C, 1], F32, kind="ExternalInput").ap()
    out = nc.dram_tensor("out", [C, TOK], F32, kind="ExternalOutput").ap()

    with tile.TileContext(nc) as tc:
        with tc.tile_pool(name="dram", bufs=1, space="DRAM") as dram:
            k_bounce = dram.tile([C, TOK], BF)
            v_bounce = dram.tile([TOK, C], BF)
            k_all = dram.tile([N_CORES * C, TOK], BF, addr_space="Shared")
            v_all = dram.tile([N_CORES * TOK, C], BF, addr_space="Shared")

            with tc.tile_pool(name="persist", bufs=1) as persist:
                qT_sb = persist.tile([128, CT, TOK], BF)
                attn_sb = persist.tile([128, CT, TOK], BF)
                ones_sb = persist.tile([128, 64], BF)
                nc.vector.memset(ones_sb[:], 1.0)
                wo_sb = persist.tile([128, CT, C], BF)
                bo_sb = persist.tile([128, CT, 1], F32)

                with tc.tile_pool(name="psum", bufs=1, space="PSUM") as psum:

                    # ---------------- Phase 1: projections ----------------
                    with tc.tile_pool(name="p1", bufs=1) as p1:
                        xT_sb = p1.tile([128, CT, TOK], BF)
                        nc.sync.dma_start(
                            xT_sb[:], xT.rearrange("(n p) t -> p n t", p=128))
                        wk_sb = p1.tile([128, CT, C], BF)
                        nc.sync.dma_start(
                            wk_sb[:], wkT.rearrange("(n p) o -> p n o", p=128))
                        wv_sb = p1.tile([128, CT, C], BF)
                        nc.sync.dma_start(
                            wv_sb[:], wvT.rearrange("(n p) o -> p n o", p=128))
                        wq_sb = p1.tile([128, CT, C], BF)
                        nc.sync.dma_start(
                            wq_sb[:], wqT.rearrange("(n p) o -> p n o", p=128))

                        def proj_T(w_sb, dram_dst, sbuf_dst):
                            # out[o, t] = sum_i W^T[i, o] x^T[i, t]
                            for ot in range(CT):
                                for t0, tsz in _chunks(TOK, 512):
                                    ps = psum.tile([128, 512], F32, tag="mm",
                                                   bufs=4, name="ps_proj")
                                    for i in range(CT):
                                        nc.tensor.matmul(
                                            ps[:, :tsz],
                                            w_sb[:, i, ot * 128:(ot + 1) * 128],
                                            xT_sb[:, i, t0:t0 + tsz],
                                            start=(i == 0), stop=(i == CT - 1))
                                    if sbuf_dst is not None:
                                        nc.vector.tensor_copy(
                                            sbuf_dst[:, ot, t0:t0 + tsz],
                                            ps[:, :tsz])
                                    else:
                                        st = p1.tile([128, 512], BF, tag="st",
                                                     bufs=4, name="st_proj")
                                        nc.vector.tensor_copy(st[:, :tsz],
                                                              ps[:, :tsz])
                                        nc.sync.dma_start(
                                            dram_dst[ot * 128:(ot + 1) * 128,
                                                     t0:t0 + tsz],
                                            st[:, :tsz])

                        # K^T -> bounce -> AllGather
                        proj_T(wk_sb, k_bounce, None)
                        nc.gpsimd.collective_compute(
                            "AllGather", mybir.AluOpType.bypass,
                            replica_groups=[list(range(N_CORES))],
                            ins=[k_bounce[:].opt()],
                            outs=[k_all[:].opt()])

                        # V (token-major) -> bounce -> AllGather
                        for tt in range(TOK // 128):
                            for o0, osz in _chunks(C, 512):
                                ps = psum.tile([128, 512], F32, tag="mm",
                                               bufs=4, name="ps_v")
                                for i in range(CT):
                                    nc.tensor.matmul(
                                        ps[:, :osz],
                                        xT_sb[:, i, tt * 128:(tt + 1) * 128],
                                        wv_sb[:, i, o0:o0 + osz],
                                        start=(i == 0), stop=(i == CT - 1))
                                st = p1.tile([128, 512], BF, tag="st",
                                             bufs=4, name="st_v")
                                nc.vector.tensor_copy(st[:, :osz], ps[:, :osz])
                                nc.sync.dma_start(
                                    v_bounce[tt * 128:(tt + 1) * 128,
                                             o0:o0 + osz],
                                    st[:, :osz])
                        nc.gpsimd.collective_compute(
                            "AllGather", mybir.AluOpType.bypass,
                            replica_groups=[list(range(N_CORES))],
                            ins=[v_bounce[:].opt()],
                            outs=[v_all[:].opt()])

                        # Q^T kept in SBUF
                        proj_T(wq_sb, None, qT_sb)

                        nc.sync.dma_start(
                            wo_sb[:], woT.rearrange("(n p) o -> p n o", p=128))
                        nc.sync.dma_start(
                            bo_sb[:], bo_d.rearrange("(n p) o -> p n o", p=128))

                    # ------------- Phase 2+3: attention + out-proj -------------
                    with tc.tile_pool(name="p2", bufs=1) as p2:
                        k_all_v = k_all[:].rearrange("(s r) t -> r s t",
                                                     s=N_CORES)
                        for b in range(B):
                            for hp in range(CT):
                                kp = p2.tile([128, N_CORES, TS], BF, tag="kp",
                                             bufs=2, name="kp")
                                nc.sync.dma_start(
                                    kp[:],
                                    k_all_v[hp * 128:(hp + 1) * 128, :,
                                            b * TS:(b + 1) * TS])
                                kp_f = kp[:].rearrange("p s t -> p (s t)")

                                vp = p2.tile([128, KT, 130], BF, tag="vp",
                                             bufs=2, name="vp")
                                nc.vector.memset(vp[:], 1.0)
                                for kt in range(KT):
                                    # global key kt*128+p lives at rank s,
                                    # batch b, local token offset ofs
                                    g0 = kt * 128
                                    s = g0 // TS
                                    ofs = g0 % TS
                                    src = v_all[s * TOK + b * TS + ofs:
                                                s * TOK + b * TS + ofs + 128,
                                                hp * 128:(hp + 1) * 128]
                                    nc.sync.dma_start(
                                        vp[:, kt, :].rearrange(
                                            "p (h c) -> p h c", h=2)[:, :, 0:64],
                                        src.rearrange("p (h c) -> p h c", h=2))

                                # S_T matmuls, two heads interleaved for PE
                                # row-group concurrency (K=64 each)
                                stg0 = p2.tile([128, KT * TS], F32,
                                               tag="stage", bufs=3, name="stg0")
                                stg1 = p2.tile([128, KT * TS], F32,
                                               tag="stage", bufs=3, name="stg1")
                                for kt2 in range(KT // 2):
                                    ps0 = psum.tile([128, 512], F32, tag="mm",
                                                    bufs=4, name="ps_s0")
                                    ps1 = psum.tile([128, 512], F32, tag="mm",
                                                    bufs=4, name="ps_s1")
                                    for j in (0, 1):
                                        kt = kt2 * 2 + j
                                        for h, ps in ((0, ps0), (1, ps1)):
                                            nc.tensor.matmul(
                                                ps[:, j * TS:(j + 1) * TS],
                                                kp_f[h * 64:(h + 1) * 64,
                                                     kt * 128:(kt + 1) * 128],
                                                qT_sb[h * 64:(h + 1) * 64, hp,
                                                      b * TS:(b + 1) * TS],
                                                start=True, stop=True,
                                                tile_position=(h * 64, 0))
                                    nc.vector.tensor_copy(
                                        stg0[:, kt2 * 2 * TS:(kt2 + 1) * 2 * TS],
                                        ps0[:, :2 * TS])
                                    nc.vector.tensor_copy(
                                        stg1[:, kt2 * 2 * TS:(kt2 + 1) * 2 * TS],
                                        ps1[:, :2 * TS])

                                for h, stg in ((0, stg0), (1, stg1)):
                                    P = p2.tile([128, KT * TS], BF, tag="P",
                                                bufs=3, name="P")
                                    nc.scalar.activation(
                                        P[:], stg[:],
                                        mybir.ActivationFunctionType.Exp,
                                        scale=SCALE)
                                    pav = psum.tile([65, TS], F32, tag="pav",
                                                    bufs=2, name="pav")
                                    for kt in range(KT):
                                        nc.tensor.matmul(
                                            pav[:],
                                            vp[:, kt, h * 65:(h + 1) * 65],
                                            P[:, kt * TS:(kt + 1) * TS],
                                            start=(kt == 0),
                                            stop=(kt == KT - 1))
                                    recip = p2.tile([128, TS], BF, tag="recip",
                                                    bufs=2, name="recip")
                                    with nc.allow_low_precision(
                                            reason="softmax denom in bf16"):
                                        nc.vector.reciprocal(recip[64:65, :],
                                                             pav[64:65, :])
                                    pbc = psum.tile([64, TS], F32, tag="pbc",
                                                    bufs=2, name="pbc")
                                    nc.tensor.matmul(
                                        pbc[:], ones_sb[64:65, :],
                                        recip[64:65, :],
                                        start=True, stop=True)
                                    bcast_sb = p2.tile([64, TS], F32,
                                                       tag="bcast", bufs=2,
                                                       name="bcast_sb")
                                    nc.vector.tensor_copy(bcast_sb[:], pbc[:])
                                    tmp = p2.tile([64, TS], BF, tag="tmp",
                                                  bufs=3, name="tmp")
                                    nc.vector.tensor_mul(tmp[:], pav[0:64, :],
                                                         bcast_sb[:])
                                    nc.sync.dma_start(
                                        attn_sb[h * 64:(h + 1) * 64, hp,
                                                b * TS:(b + 1) * TS],
                                        tmp[:])

                            # out projection for this batch's tokens
                            for co in range(CT):
                                psy = psum.tile([128, TS], F32, tag="mm",
                                                bufs=4, name="psy")
                                for ct in range(CT):
                                    nc.tensor.matmul(
                                        psy[:],
                                        wo_sb[:, ct, co * 128:(co + 1) * 128],
                                        attn_sb[:, ct, b * TS:(b + 1) * TS],
                                        start=(ct == 0), stop=(ct == CT - 1))
                                ysb = p2.tile([128, TS], F32, tag="y", bufs=3,
                                              name="ysb")
                                nc.vector.tensor_scalar_add(
                                    ysb[:], psy[:], bo_sb[:, co, :])
                                nc.sync.dma_start(
                                    out[co * 128:(co + 1) * 128,
                                        b * TS:(b + 1) * TS],
                                    ysb[:])

    nc.compile()
    return nc


def _prep_inputs(hidden_states, Wq, Wk, Wv, Wo, bo):
    T = hidden_states.shape[1]
    TS = T // N_CORES
    TOK = B * TS
    bf = ml_dtypes.bfloat16
    wqT = np.ascontiguousarray(np.asarray(Wq, np.float32).T).astype(bf)
    wkT = np.ascontiguousarray(np.asarray(Wk, np.float32).T).astype(bf)
    wvT = np.ascontiguousarray(np.asarray(Wv, np.float32).T).astype(bf)
    woT = np.ascontiguousarray(np.asarray(Wo, np.float32).T).astype(bf)
    bo_c = np.asarray(bo, np.float32).reshape(C, 1)
    x = np.asarray(hidden_states, np.float32)
    in_maps = []
    for r in range(N_CORES):
        xr = x[:, r * TS:(r + 1) * TS, :].reshape(TOK, C)
        xT = np.ascontiguousarray(xr.T).astype(bf)
        in_maps.append({
            "xT": xT, "wqT": wqT, "wkT": wkT, "wvT": wvT, "woT": woT,
            "bo": bo_c,
        })
    return in_maps


def kernel(hidden_states, Wq, Wk, Wv, Wo, bo):
    global LAST_EXEC_TIME_NS
    _install_ntff_hook()
    Bx, T, Cx = hidden_states.shape
    assert (Bx, Cx) == (B, C)
    TS = T // N_CORES
    TOK = B * TS
    if T not in _BUILD_CACHE:
        _BUILD_CACHE[T] = build(T)
    nc = _BUILD_CACHE[T]
    in_maps = _prep_inputs(hidden_states, Wq, Wk, Wv, Wo, bo)
    res = run_bass_kernel_spmd(nc, in_maps, core_ids=list(range(N_CORES)))
    LAST_EXEC_TIME_NS = res.exec_time_ns
    outf = np.empty((B, T, C), np.float32)
    for r in range(N_CORES):
        yT = res.results[r]["out"]          # [C, TOK]
        yr = yT.T.reshape(B, TS, C)
        outf[:, r * TS:(r + 1) * TS, :] = yr
    return outf


# revision 6
# speedup vs baseline: 1.0790x; 1.0790x over previous
"""Distributed multi-head attention kernel for 8 Trainium2 NeuronCores.

Problem: y = softmax((x Wq^T)(x Wk^T)^T / sqrt(D)) (x Wv^T) Wo^T + bo
with B=4, T=2048, C=1280, H=20, D=64, float32 I/O.

Sharding (sequence parallel, rank independent):
  Each core owns a T/8 token slice of all 4 batches (1024 tokens).
  It computes Q/K/V projections for its tokens, AllGathers K^T and V
  (bf16) across the 8 cores, runs full attention for its queries over
  the gathered keys/values of the matching batch, and applies the
  output projection for its tokens. Host reassembles the T axis.

Attention runs in a transposed "S_T[k, q]" layout so the softmax
denominator falls out of the same matmul that computes P@V: V is
stored padded per head as [.., 65] with a constant-1 column, so psum
row 64 of the P@V accumulation is sum_k P. This avoids partition-axis
reductions entirely. exp() runs on ScalarE directly out of PSUM in
[128, 4*TS] chunks; normalization is a reciprocal + a K=1 broadcast
matmul + one vector multiply.

Compute dtype is bf16 (fp32 matmul is 4x slower on the PE array);
accumulation is fp32 in PSUM. I/O stays fp32.
"""

import os
import sys
import types

import numpy as np
import ml_dtypes

import concourse.bass as bass
import concourse.mybir as mybir
import concourse.tile as tile
from concourse import bacc
from concourse.bass_utils import run_bass_kernel_spmd

N_CORES = 8
C = 1280
H = 20
D = 64
B = 4
CT = C // 128  # 10 c-tiles
HP5 = H * 65   # padded V row width (64 dims + ones column per head)
BF = mybir.dt.bfloat16
F32 = mybir.dt.float32
SCALE = 1.0 / (D ** 0.5)

LAST_EXEC_TIME_NS = None
_BUILD_CACHE = {}


def _install_ntff_hook():
    """The trimmed antenv package lacks axon_hooks; register the NTFF
    profile hook by hand so trace=True can time the NEFF on silicon.
    Safe no-op if anything is missing."""
    if "antenv.axon_hooks" in sys.modules:
        return
    try:
        from trn_agent_boot.trn_boot import _ntff_profile_via_ctypes

        hook = _ntff_profile_via_ctypes("/opt/axon/libaxon_pjrt.so")
        mod = types.ModuleType("antenv.axon_hooks")
        mod.get_axon_ntff_profile_hook = lambda: hook
        mod.set_axon_ntff_profile_hook = lambda h: None
        sys.modules["antenv.axon_hooks"] = mod
        import antenv

        antenv.axon_hooks = mod
    except Exception:
        pass


def _chunks(total, step):
    out = []
    o = 0
    while o < total:
        out.append((o, min(step, total - o)))
        o += step
    return out


def build(T):
    """Build the SPMD Bass graph for full (unsharded) sequence length T."""
    TS = T // N_CORES          # tokens per batch per core
    TOK = B * TS               # tokens per core
    KT = T // 128              # 128-wide key tiles per batch
    assert TS % 128 == 0, "key tiles must not cross rank chunks"
    assert TS <= 512, "q tile must fit one matmul moving operand"
    assert KT % 4 == 0
    C4 = KT // 4               # exp chunks (4 key tiles each) per head

    nc = bacc.Bacc("TRN2", target_bir_lowering=False, debug=False,
                   num_devices=N_CORES)

    xT = nc.dram_tensor("xT", [C, TOK], BF, kind="ExternalInput").ap()
    wqT = nc.dram_tensor("wqT", [C, C], BF, kind="ExternalInput").ap()
    wkT = nc.dram_tensor("wkT", [C, C], BF, kind="ExternalInput").ap()
    wvT = nc.dram_tensor("wvT", [C, C], BF, kind="ExternalInput").ap()
    woT = nc.dram_tensor("woT", [C, C], BF, kind="ExternalInput").ap()
    bo_d = nc.dram_tensor("bo", [C, 1], F32, kind="ExternalInput").ap()
    out = nc.dram_tensor("out", [C, TOK], F32, kind="ExternalOutput").ap()

    with tile.TileContext(nc) as tc:
        with tc.tile_pool(name="dram", bufs=1, space="DRAM") as dram:
            k_bounce = dram.tile([C, TOK], BF)
            v_bounce = dram.tile([TOK, HP5], BF)
            k_all = dram.tile([N_CORES * C, TOK], BF, addr_space="Shared")
            v_all = dram.tile([N_CORES * TOK, HP5], BF, addr_space="Shared")

            with tc.tile_pool(name="persist", bufs=1) as persist:
                qT_sb = persist.tile([128, CT, TOK], BF)
                attn_sb = persist.tile([128, CT, TOK], BF)
                ones_sb = persist.tile([128, 64], BF)
                nc.vector.memset(ones_sb[:], 1.0)
                wo_sb = persist.tile([128, CT, C], BF)
                bo_sb = persist.tile([128, CT, 1], F32)

                # ---------------- Phase 1: projections ----------------
                with tc.tile_pool(name="p1", bufs=1) as p1, \
                     tc.tile_pool(name="psum1", bufs=1, space="PSUM") as psum1:
                    xT_sb = p1.tile([128, CT, TOK], BF)
                    nc.sync.dma_start(
                        xT_sb[:], xT.rearrange("(n p) t -> p n t", p=128))
                    wk_sb = p1.tile([128, CT, C], BF)
                    nc.sync.dma_start(
                        wk_sb[:], wkT.rearrange("(n p) o -> p n o", p=128))
                    wv_sb = p1.tile([128, CT, C], BF)
                    nc.sync.dma_start(
                        wv_sb[:], wvT.rearrange("(n p) o -> p n o", p=128))
                    wq_sb = p1.tile([128, CT, C], BF)
                    nc.sync.dma_start(
                        wq_sb[:], wqT.rearrange("(n p) o -> p n o", p=128))

                    def proj_T(w_sb, dram_dst, sbuf_dst):
                        # out[o, t] = sum_i W^T[i, o] x^T[i, t]
                        for ot in range(CT):
                            for t0, tsz in _chunks(TOK, 512):
                                ps = psum1.tile([128, 512], F32, tag="mm",
                                                bufs=6, name="ps_proj")
                                for i in range(CT):
                                    nc.tensor.matmul(
                                        ps[:, :tsz],
                                        w_sb[:, i, ot * 128:(ot + 1) * 128],
                                        xT_sb[:, i, t0:t0 + tsz],
                                        start=(i == 0), stop=(i == CT - 1))
                                if sbuf_dst is not None:
                                    nc.vector.tensor_copy(
                                        sbuf_dst[:, ot, t0:t0 + tsz],
                                        ps[:, :tsz])
                                else:
                                    st = p1.tile([128, 512], BF, tag="st",
                                                 bufs=4, name="st_proj")
                                    nc.vector.tensor_copy(st[:, :tsz],
                                                          ps[:, :tsz])
                                    nc.sync.dma_start(
                                        dram_dst[ot * 128:(ot + 1) * 128,
                                                 t0:t0 + tsz],
                                        st[:, :tsz])

                    # K^T -> bounce -> AllGather
                    proj_T(wk_sb, k_bounce, None)
                    nc.gpsimd.collective_compute(
                        "AllGather", mybir.AluOpType.bypass,
                        replica_groups=[list(range(N_CORES))],
                        ins=[k_bounce[:].opt()],
                        outs=[k_all[:].opt()])

                    # V (token-major, per-head 65-padded with a ones col)
                    for tt in range(TOK // 128):
                        stv = p1.tile([128, H, 65], BF, tag="stv", bufs=2,
                                      name="stv")
                        nc.vector.memset(stv[:, :, 64:65], 1.0)
                        for o0, osz in _chunks(C, 512):
                            ps = psum1.tile([128, 512], F32, tag="mm",
                                            bufs=6, name="ps_v")
                            for i in range(CT):
                                nc.tensor.matmul(
                                    ps[:, :osz],
                                    xT_sb[:, i, tt * 128:(tt + 1) * 128],
                                    wv_sb[:, i, o0:o0 + osz],
                                    start=(i == 0), stop=(i == CT - 1))
                            h0, nh = o0 // 64, osz // 64
                            nc.vector.tensor_copy(
                                stv[:, h0:h0 + nh, 0:64],
                                ps[:, :osz].rearrange("p (h c) -> p h c",
                                                      c=64))
                        nc.sync.dma_start(
                            v_bounce[tt * 128:(tt + 1) * 128, :],
                            stv[:].rearrange("p h c -> p (h c)"))
                    nc.gpsimd.collective_compute(
                        "AllGather", mybir.AluOpType.bypass,
                        replica_groups=[list(range(N_CORES))],
                        ins=[v_bounce[:].opt()],
                        outs=[v_all[:].opt()])

                    # Q^T kept in SBUF
                    proj_T(wq_sb, None, qT_sb)

                    nc.sync.dma_start(
                        wo_sb[:], woT.rearrange("(n p) o -> p n o", p=128))
                    nc.sync.dma_start(
                        bo_sb[:], bo_d.rearrange("(n p) o -> p n o", p=128))

                # ------------- Phase 2+3: attention + out-proj -------------
                with tc.tile_pool(name="p2", bufs=1) as p2, \
                     tc.tile_pool(name="psum2", bufs=1, space="PSUM") as psum2:
                    k_all_v = k_all[:].rearrange("(s r) t -> r s t",
                                                 s=N_CORES)
                    # v rows for batch b, ordered by global key index:
                    # key g at rank g // TS, local row b*TS + g % TS
                    v_all_v = v_all[:].rearrange("(s j p) c -> s j p c",
                                                 s=N_CORES, p=128)
                    JR = TS // 128  # key tiles per rank chunk

                    for b in range(B):
                        # V for this batch, both halves double-buffered
                        vbs = []
                        for half in range(2):
                            vb = p2.tile([128, KT // 2, HP5], BF, tag="vb",
                                         bufs=2, name=f"vb{half}")
                            for s0 in range(N_CORES // 2):
                                s = half * (N_CORES // 2) + s0
                                nc.sync.dma_start(
                                    vb[:, s0 * JR:(s0 + 1) * JR, :],
                                    v_all_v[s, b * JR:(b + 1) * JR, :, :]
                                    .rearrange("j p c -> p j c"))
                            vbs.append(vb)

                        for hp in range(CT):
                            kp = p2.tile([128, N_CORES, TS], BF, tag="kp",
                                         bufs=2, name="kp")
                            nc.sync.dma_start(
                                kp[:],
                                k_all_v[hp * 128:(hp + 1) * 128, :,
                                        b * TS:(b + 1) * TS])
                            kp_f = kp[:].rearrange("p s t -> p (s t)")

                            P0 = p2.tile([128, KT * TS], BF, tag="P",
                                         bufs=3, name="P0")
                            P1 = p2.tile([128, KT * TS], BF, tag="P",
                                         bufs=3, name="P1")
                            for c4 in range(C4):
                                psA = psum2.tile([128, 4 * TS], F32,
                                                 tag="ps_s", bufs=2,
                                                 name="psA")
                                psB = psum2.tile([128, 4 * TS], F32,
                                                 tag="ps_s", bufs=2,
                                                 name="psB")
                                for j in range(4):
                                    kt = c4 * 4 + j
                                    for h, ps in ((0, psA), (1, psB)):
                                        nc.tensor.matmul(
                                            ps[:, j * TS:(j + 1) * TS],
                                            kp_f[h * 64:(h + 1) * 64,
                                                 kt * 128:(kt + 1) * 128],
                                            qT_sb[h * 64:(h + 1) * 64, hp,
                                                  b * TS:(b + 1) * TS],
                                            start=True, stop=True,
                                            tile_position=(h * 64, 0))
                                for ps, P in ((psA, P0), (psB, P1)):
                                    nc.scalar.activation(
                                        P[:, c4 * 4 * TS:(c4 + 1) * 4 * TS],
                                        ps[:],
                                        mybir.ActivationFunctionType.Exp,
                                        scale=SCALE)

                            for h, P in ((0, P0), (1, P1)):
                                hg = 2 * hp + h
                                pav = psum2.tile([65, TS], F32, tag="pav",
                                                 bufs=2, name="pav")
                                for kt in range(KT):
                                    vb = vbs[0] if kt < KT // 2 else vbs[1]
                                    ktl = kt % (KT // 2)
                                    nc.tensor.matmul(
                                        pav[:],
                                        vb[:, ktl, hg * 65:(hg + 1) * 65],
                                        P[:, kt * TS:(kt + 1) * TS],
                                        start=(kt == 0),
                                        stop=(kt == KT - 1))
                                recip = p2.tile([128, TS], BF, tag="recip",
                                                bufs=2, name="recip")
                                with nc.allow_low_precision(
                                        reason="softmax denom in bf16"):
                                    nc.vector.reciprocal(recip[64:65, :],
                                                         pav[64:65, :])
                                pbc = psum2.tile([64, TS], F32, tag="pbc",
                                                 bufs=1, name="pbc")
                                nc.tensor.matmul(
                                    pbc[:], ones_sb[64:65, :],
                                    recip[64:65, :],
                                    start=True, stop=True)
                                bcast_sb = p2.tile([64, TS], F32,
                                                   tag="bcast", bufs=2,
                                                   name="bcast_sb")
                                nc.vector.tensor_copy(bcast_sb[:], pbc[:])
                                tmp = p2.tile([64, TS], BF, tag="tmp",
                                              bufs=3, name="tmp")
                                nc.vector.tensor_mul(tmp[:], pav[0:64, :],
                                                     bcast_sb[:])
                                nc.sync.dma_start(
                                    attn_sb[h * 64:(h + 1) * 64, hp,
                                            b * TS:(b + 1) * TS],
                                    tmp[:])

                        # out projection for this batch's tokens
                        for co in range(CT):
                            psy = psum2.tile([128, TS], F32, tag="psy",
                                             bufs=1, name="psy")
                            for ct in range(CT):
                                nc.tensor.matmul(
                                    psy[:],
                                    wo_sb[:, ct, co * 128:(co + 1) * 128],
                                    attn_sb[:, ct, b * TS:(b + 1) * TS],
                                    start=(ct == 0), stop=(ct == CT - 1))
                            ysb = p2.tile([128, TS], F32, tag="y", bufs=3,
                                          name="ysb")
                            nc.vector.tensor_scalar_add(
                                ysb[:], psy[:], bo_sb[:, co, :])
                            nc.sync.dma_start(
                                out[co * 128:(co + 1) * 128,
                                    b * TS:(b + 1) * TS],
                                ysb[:])

    nc.compile()
    return nc


def _prep_inputs(hidden_states, Wq, Wk, Wv, Wo, bo):
    T = hidden_states.shape[1]
    TS = T // N_CORES
    TOK = B * TS
    bf = ml_dtypes.bfloat16
    wqT = np.ascontiguousarray(np.asarray(Wq, np.float32).T).astype(bf)
    wkT = np.ascontiguousarray(np.asarray(Wk, np.float32).T).astype(bf)
    wvT = np.ascontiguousarray(np.asarray(Wv, np.float32).T).astype(bf)
    woT = np.ascontiguousarray(np.asarray(Wo, np.float32).T).astype(bf)
    bo_c = np.asarray(bo, np.float32).reshape(C, 1)
    x = np.asarray(hidden_states, np.float32)
    in_maps = []
    for r in range(N_CORES):
        xr = x[:, r * TS:(r + 1) * TS, :].reshape(TOK, C)
        xT = np.ascontiguousarray(xr.T).astype(bf)
        in_maps.append({
            "xT": xT, "wqT": wqT, "wkT": wkT, "wvT": wvT, "woT": woT,
            "bo": bo_c,
        })
    return in_maps


def kernel(hidden_states, Wq, Wk, Wv, Wo, bo):
    global LAST_EXEC_TIME_NS
    _install_ntff_hook()
    Bx, T, Cx = hidden_states.shape
    assert (Bx, Cx) == (B, C)
    TS = T // N_CORES
    if T not in _BUILD_CACHE:
        _BUILD_CACHE[T] = build(T)
    nc = _BUILD_CACHE[T]
    in_maps = _prep_inputs(hidden_states, Wq, Wk, Wv, Wo, bo)
    res = run_bass_kernel_spmd(nc, in_maps, core_ids=list(range(N_CORES)))
    LAST_EXEC_TIME_NS = res.exec_time_ns
    outf = np.empty((B, T, C), np.float32)
    for r in range(N_CORES):
        yT = res.results[r]["out"]          # [C, TOK]
        yr = yT.T.reshape(B, TS, C)
        outf[:, r * TS:(r + 1) * TS, :] = yr
    return outf


# revision 14
# speedup vs baseline: 1.1157x; 1.0340x over previous
"""Distributed multi-head attention kernel for 8 Trainium2 NeuronCores.

Problem: y = softmax((x Wq^T)(x Wk^T)^T / sqrt(D)) (x Wv^T) Wo^T + bo
with B=4, T=2048, C=1280, H=20, D=64, float32 I/O.

Sharding (sequence parallel, rank independent):
  Each core owns a T/8 token slice of all 4 batches (1024 tokens).
  It computes Q/K/V projections for its tokens, AllGathers K^T and V
  (bf16, chunked per batch pair so comm overlaps compute), runs full
  attention for its queries over the gathered keys/values of the
  matching batch, and applies the output projection for its tokens.
  The host reassembles the T axis.

Attention runs in a transposed "S_T[k, q]" layout so the softmax
denominator falls out of the same matmul that computes P@V: V is
stored padded per head as [.., 65] with a constant-1 column, so psum
row 64 of the P@V accumulation is sum_k P. This avoids partition-axis
reductions entirely. exp() runs on ScalarE directly out of PSUM in
[128, 4*TS] chunks; P@V is interleaved per chunk so the PE keeps
working while ScalarE drains. Q-projection for batches 2-3 is emitted
after batch-0 attention as PE gap filler.

Compute dtype is bf16 (fp32 matmul is 4x slower on the PE array);
accumulation is fp32 in PSUM. I/O stays fp32.
"""

import os
import sys
import types

import numpy as np
import ml_dtypes

import concourse.bass as bass
import concourse.mybir as mybir
import concourse.tile as tile
from concourse import bacc
from concourse.bass_utils import run_bass_kernel_spmd

N_CORES = 8
C = 1280
H = 20
D = 64
B = 4
CT = C // 128  # 10 c-tiles
HP5 = H * 65   # padded V row width (64 dims + ones column per head)
BF = mybir.dt.bfloat16
F32 = mybir.dt.float32
SCALE = 1.0 / (D ** 0.5)

LAST_EXEC_TIME_NS = None
_BUILD_CACHE = {}


def _install_ntff_hook():
    """The trimmed antenv package lacks axon_hooks; register the NTFF
    profile hook by hand so trace=True can time the NEFF on silicon.
    Safe no-op if anything is missing."""
    if "antenv.axon_hooks" in sys.modules:
        return
    try:
        from trn_agent_boot.trn_boot import _ntff_profile_via_ctypes

        hook = _ntff_profile_via_ctypes("/opt/axon/libaxon_pjrt.so")
        mod = types.ModuleType("antenv.axon_hooks")
        mod.get_axon_ntff_profile_hook = lambda: hook
        mod.set_axon_ntff_profile_hook = lambda h: None
        sys.modules["antenv.axon_hooks"] = mod
        import antenv

        antenv.axon_hooks = mod
    except Exception:
        pass


def _chunks(total, step):
    out = []
    o = 0
    while o < total:
        out.append((o, min(step, total - o)))
        o += step
    return out


def build(T):
    """Build the SPMD Bass graph for full (unsharded) sequence length T."""
    TS = T // N_CORES          # tokens per batch per core
    TOK = B * TS               # tokens per core
    KT = T // 128              # 128-wide key tiles per batch
    assert TS % 128 == 0, "key tiles must not cross rank chunks"
    assert TS <= 512, "q tile must fit one matmul moving operand"
    assert KT % 4 == 0
    C4 = KT // 4               # exp chunks (4 key tiles each) per head
    TH = 2 * TS                # tokens per batch pair per core
    RG = [list(range(N_CORES))]

    nc = bacc.Bacc("TRN2", target_bir_lowering=False, debug=False,
                   num_devices=N_CORES)

    xT = nc.dram_tensor("xT", [C, TOK], BF, kind="ExternalInput").ap()
    wqT = nc.dram_tensor("wqT", [C, C], BF, kind="ExternalInput").ap()
    wkT = nc.dram_tensor("wkT", [C, C], BF, kind="ExternalInput").ap()
    wvT = nc.dram_tensor("wvT", [C, C], BF, kind="ExternalInput").ap()
    woT = nc.dram_tensor("woT", [C, C], BF, kind="ExternalInput").ap()
    bo_d = nc.dram_tensor("bo", [C, 1], F32, kind="ExternalInput").ap()
    out = nc.dram_tensor("out", [C, TOK], F32, kind="ExternalOutput").ap()

    with tile.TileContext(nc) as tc:
        with tc.tile_pool(name="dram", bufs=1, space="DRAM") as dram:
            # per batch-pair bounce + gathered buffers
            k_bn = [dram.tile([C, TH], BF, name=f"k_bn{i}") for i in range(2)]
            v_bn = [dram.tile([TH, HP5], BF, name=f"v_bn{i}") for i in range(2)]
            k_all = [dram.tile([N_CORES * C, TH], BF, addr_space="Shared",
                               name=f"k_all{i}") for i in range(2)]
            v_all = [dram.tile([N_CORES * TH, HP5], BF, addr_space="Shared",
                               name=f"v_all{i}") for i in range(2)]

            with tc.tile_pool(name="persist", bufs=1) as persist, \
                 tc.tile_pool(name="p1q", bufs=1) as p1q:
                qT_sb = persist.tile([128, CT, TOK], BF)
                attn_sb = persist.tile([128, CT, TOK], BF)
                ones_sb = persist.tile([128, 64], BF)
                nc.vector.memset(ones_sb[:], 1.0)
                wo_sb = persist.tile([128, CT, C], BF)
                bo_sb = persist.tile([128, CT, 1], F32)

                xT_sb = p1q.tile([128, CT, TOK], BF)
                nc.sync.dma_start(
                    xT_sb[:], xT.rearrange("(n p) t -> p n t", p=128))
                wq_sb = p1q.tile([128, CT, C], BF)
                nc.sync.dma_start(
                    wq_sb[:], wqT.rearrange("(n p) o -> p n o", p=128))

                def proj_T(psum, w_sb, dram_dst, sbuf_dst, t_lo, t_hi, pool,
                           dst_ofs=0, psum_tag="mm", psum_bufs=6):
                    # out[o, t] = sum_i W^T[i, o] x^T[i, t]
                    for ot in range(CT):
                        for t0, tsz in _chunks(t_hi - t_lo, 512):
                            t0 += t_lo
                            ps = psum.tile([128, 512], F32, tag=psum_tag,
                                           bufs=psum_bufs, name="ps_proj")
                            for i in range(CT):
                                nc.tensor.matmul(
                                    ps[:, :tsz],
                                    w_sb[:, i, ot * 128:(ot + 1) * 128],
                                    xT_sb[:, i, t0:t0 + tsz],
                                    start=(i == 0), stop=(i == CT - 1))
                            if sbuf_dst is not None:
                                nc.vector.tensor_copy(
                                    sbuf_dst[:, ot, t0:t0 + tsz],
                                    ps[:, :tsz])
                            else:
                                st = pool.tile([128, 512], BF, tag="st",
                                               bufs=4, name="st_proj")
                                nc.vector.tensor_copy(st[:, :tsz],
                                                      ps[:, :tsz])
                                nc.sync.dma_start(
                                    dram_dst[ot * 128:(ot + 1) * 128,
                                             t0 - dst_ofs:
                                             t0 - dst_ofs + tsz],
                                    st[:, :tsz])

                def proj_V(psum, half, pool):
                    # V (token-major, per-head 65-padded with a ones col)
                    for ttl in range(TH // 128):
                        tt = half * (TH // 128) + ttl
                        stv = pool.tile([128, H, 65], BF, tag="stv", bufs=2,
                                        name="stv")
                        nc.vector.memset(stv[:, :, 64:65], 1.0)
                        for o0, osz in _chunks(C, 512):
                            ps = psum.tile([128, 512], F32, tag="mm",
                                           bufs=6, name="ps_v")
                            for i in range(CT):
                                nc.tensor.matmul(
                                    ps[:, :osz],
                                    xT_sb[:, i, tt * 128:(tt + 1) * 128],
                                    wv_sb[:, i, o0:o0 + osz],
                                    start=(i == 0), stop=(i == CT - 1))
                            h0, nh = o0 // 64, osz // 64
                            nc.vector.tensor_copy(
                                stv[:, h0:h0 + nh, 0:64],
                                ps[:, :osz].rearrange("p (h c) -> p h c",
                                                      c=64))
                        nc.sync.dma_start(
                            v_bn[half][ttl * 128:(ttl + 1) * 128, :],
                            stv[:].rearrange("p h c -> p (h c)"))

                # ---------------- Phase 1: K/V projections + AGs ----------
                with tc.tile_pool(name="p1", bufs=1) as p1, \
                     tc.tile_pool(name="psum1", bufs=1, space="PSUM") as psum1:
                    wk_sb = p1.tile([128, CT, C], BF)
                    nc.sync.dma_start(
                        wk_sb[:], wkT.rearrange("(n p) o -> p n o", p=128))
                    wv_sb = p1.tile([128, CT, C], BF)
                    nc.sync.dma_start(
                        wv_sb[:], wvT.rearrange("(n p) o -> p n o", p=128))

                    for half in range(2):
                        proj_T(psum1, wk_sb, k_bn[half], None,
                               half * TH, (half + 1) * TH, p1,
                               dst_ofs=half * TH)
                        nc.gpsimd.collective_compute(
                            "AllGather", mybir.AluOpType.bypass,
                            replica_groups=RG,
                            ins=[k_bn[half][:].opt()],
                            outs=[k_all[half][:].opt()])
                        proj_V(psum1, half, p1)
                        nc.gpsimd.collective_compute(
                            "AllGather", mybir.AluOpType.bypass,
                            replica_groups=RG,
                            ins=[v_bn[half][:].opt()],
                            outs=[v_all[half][:].opt()])

                    # Q^T for batches 0-1 (2-3 deferred as attention filler)
                    proj_T(psum1, wq_sb, None, qT_sb, 0, TH, p1)

                    nc.sync.dma_start(
                        wo_sb[:], woT.rearrange("(n p) o -> p n o", p=128))
                    nc.sync.dma_start(
                        bo_sb[:], bo_d.rearrange("(n p) o -> p n o", p=128))

                # ------------- Phase 2+3: attention + out-proj -------------
                with tc.tile_pool(name="p2", bufs=1) as p2, \
                     tc.tile_pool(name="psum2", bufs=1, space="PSUM") as psum2:
                    JR = TS // 128  # key tiles per rank chunk

                    for b in range(B):
                        half, bb = b // 2, b % 2
                        k_all_v = k_all[half][:].rearrange(
                            "(s r) t -> r s t", s=N_CORES)
                        v_all_v = v_all[half][:].rearrange(
                            "(s j p) c -> s j p c", s=N_CORES, p=128)

                        # V for this batch, two halves of key tiles
                        vbs = []
                        for kh in range(2):
                            vb = p2.tile([128, KT // 2, HP5], BF, tag="vb",
                                         bufs=2, name=f"vb{kh}")
                            for s0 in range(N_CORES // 2):
                                s = kh * (N_CORES // 2) + s0
                                nc.sync.dma_start(
                                    vb[:, s0 * JR:(s0 + 1) * JR, :],
                                    v_all_v[s, bb * JR:(bb + 1) * JR, :, :]
                                    .rearrange("j p c -> p j c"))
                            vbs.append(vb)

                        for hp in range(CT):
                            kp = p2.tile([128, N_CORES, TS], BF, tag="kp",
                                         bufs=2, name="kp")
                            nc.sync.dma_start(
                                kp[:],
                                k_all_v[hp * 128:(hp + 1) * 128, :,
                                        bb * TS:(bb + 1) * TS])
                            kp_f = kp[:].rearrange("p s t -> p (s t)")

                            P0 = p2.tile([128, KT * TS], BF, tag="P",
                                         bufs=2, name="P0")
                            P1 = p2.tile([128, KT * TS], BF, tag="P",
                                         bufs=2, name="P1")
                            pav0 = psum2.tile([65, TS], F32, tag="pav",
                                              bufs=2, name="pav0")
                            pav1 = psum2.tile([65, TS], F32, tag="pav",
                                              bufs=2, name="pav1")
                            pavs = (pav0, pav1)
                            for c4 in range(C4):
                                psA = psum2.tile([128, 4 * TS], F32,
                                                 tag="ps_s", bufs=2,
                                                 name="psA")
                                psB = psum2.tile([128, 4 * TS], F32,
                                                 tag="ps_s", bufs=2,
                                                 name="psB")
                                for j in range(4):
                                    kt = c4 * 4 + j
                                    for h, ps in ((0, psA), (1, psB)):
                                        nc.tensor.matmul(
                                            ps[:, j * TS:(j + 1) * TS],
                                            kp_f[h * 64:(h + 1) * 64,
                                                 kt * 128:(kt + 1) * 128],
                                            qT_sb[h * 64:(h + 1) * 64, hp,
                                                  b * TS:(b + 1) * TS],
                                            start=True, stop=True,
                                            tile_position=(h * 64, 0))
                                for ps, P in ((psA, P0), (psB, P1)):
                                    nc.scalar.activation(
                                        P[:, c4 * 4 * TS:(c4 + 1) * 4 * TS],
                                        ps[:],
                                        mybir.ActivationFunctionType.Exp,
                                        scale=SCALE)
                                for h, P in ((0, P0), (1, P1)):
                                    hg = 2 * hp + h
                                    for j in range(4):
                                        kt = c4 * 4 + j
                                        vb = vbs[0] if kt < KT // 2 else vbs[1]
                                        ktl = kt % (KT // 2)
                                        nc.tensor.matmul(
                                            pavs[h][:],
                                            vb[:, ktl,
                                               hg * 65:(hg + 1) * 65],
                                            P[:, kt * TS:(kt + 1) * TS],
                                            start=(kt == 0),
                                            stop=(kt == KT - 1))

                            for h in (0, 1):
                                recip = p2.tile([128, TS], BF, tag="recip",
                                                bufs=2, name="recip")
                                with nc.allow_low_precision(
                                        reason="softmax denom in bf16"):
                                    nc.vector.reciprocal(recip[64:65, :],
                                                         pavs[h][64:65, :])
                                pbc = psum2.tile([64, TS], F32, tag="pbc",
                                                 bufs=1, name="pbc")
                                nc.tensor.matmul(
                                    pbc[:], ones_sb[64:65, :],
                                    recip[64:65, :],
                                    start=True, stop=True)
                                bcast_sb = p2.tile([64, TS], F32,
                                                   tag="bcast", bufs=2,
                                                   name="bcast_sb")
                                nc.vector.tensor_copy(bcast_sb[:], pbc[:])
                                tmp = p2.tile([64, TS], BF, tag="tmp",
                                              bufs=3, name="tmp")
                                nc.vector.tensor_mul(tmp[:], pavs[h][0:64, :],
                                                     bcast_sb[:])
                                nc.sync.dma_start(
                                    attn_sb[h * 64:(h + 1) * 64, hp,
                                            b * TS:(b + 1) * TS],
                                    tmp[:])

                        if b == 0:
                            # Q^T for batches 2-3: ready PE work that fills
                            # ScalarE-bound gaps during attention
                            proj_T(psum2, wq_sb, None, qT_sb, TH, TOK, p2,
                                   psum_tag="ps_s", psum_bufs=2)

                        # out projection for this batch's tokens
                        for co in range(CT):
                            psy = psum2.tile([128, TS], F32, tag="ps_s",
                                             bufs=2, name="psy")
                            for ct in range(CT):
                                nc.tensor.matmul(
                                    psy[:],
                                    wo_sb[:, ct, co * 128:(co + 1) * 128],
                                    attn_sb[:, ct, b * TS:(b + 1) * TS],
                                    start=(ct == 0), stop=(ct == CT - 1))
                            ysb = p2.tile([128, TS], F32, tag="y", bufs=3,
                                          name="ysb")
                            nc.vector.tensor_scalar_add(
                                ysb[:], psy[:], bo_sb[:, co, :])
                            nc.sync.dma_start(
                                out[co * 128:(co + 1) * 128,
                                    b * TS:(b + 1) * TS],
                                ysb[:])

    nc.compile()
    return nc


def _prep_inputs(hidden_states, Wq, Wk, Wv, Wo, bo):
    T = hidden_states.shape[1]
    TS = T // N_CORES
    TOK = B * TS
    bf = ml_dtypes.bfloat16
    wqT = np.ascontiguousarray(np.asarray(Wq, np.float32).T).astype(bf)
    wkT = np.ascontiguousarray(np.asarray(Wk, np.float32).T).astype(bf)
    wvT = np.ascontiguousarray(np.asarray(Wv, np.float32).T).astype(bf)
    woT = np.ascontiguousarray(np.asarray(Wo, np.float32).T).astype(bf)
    bo_c = np.asarray(bo, np.float32).reshape(C, 1)
    x = np.asarray(hidden_states, np.float32)
    in_maps = []
    for r in range(N_CORES):
        xr = x[:, r * TS:(r + 1) * TS, :].reshape(TOK, C)
        xTr = np.ascontiguousarray(xr.T).astype(bf)
        in_maps.append({
            "xT": xTr, "wqT": wqT, "wkT": wkT, "wvT": wvT, "woT": woT,
            "bo": bo_c,
        })
    return in_maps


def kernel(hidden_states, Wq, Wk, Wv, Wo, bo):
    global LAST_EXEC_TIME_NS
    _install_ntff_hook()
    Bx, T, Cx = hidden_states.shape
    assert (Bx, Cx) == (B, C)
    TS = T // N_CORES
    if T not in _BUILD_CACHE:
        _BUILD_CACHE[T] = build(T)
    nc = _BUILD_CACHE[T]
    in_maps = _prep_inputs(hidden_states, Wq, Wk, Wv, Wo, bo)
    res = run_bass_kernel_spmd(nc, in_maps, core_ids=list(range(N_CORES)))
    LAST_EXEC_TIME_NS = res.exec_time_ns
    outf = np.empty((B, T, C), np.float32)
    for r in range(N_CORES):
        yT = res.results[r]["out"]          # [C, TOK]
        yr = yT.T.reshape(B, TS, C)
        outf[:, r * TS:(r + 1) * TS, :] = yr
    return outf
